# revision 54
# baseline (speedup 1.0000x reference)
"""Fused single-head cross-attention on 8 TRN2 NeuronCores (Bass/Tile).

Problem: out = (softmax(norm * (xWq+bq)(yWk+bk)^T + adj) @ (yWv+bv)) Wo + bo
Shapes: x,y [4, 2048, 1024], adj [4, 2048, 2048], all weights [1024, 1024].

Sharding: data-parallel over (batch, seq-half) -> 8 shards. Core c handles
batch b=c//2, query rows h*1024..(h+1)*1024 (h=c%2). K/V projections are
split across the core pair (each computes its own t-half of K^T and V) and
exchanged with one pair-wise AllGather each.

v10.2 = v10 + pipeline/DMA polish (HW 210us at moderate throttle):
  * reciprocal -> reciprocal_approx_fast (0.7us vs 3.4us on DVE; 18
    correct bits vs denominators ~1e3 -- error contribution ~4e-6).
  * att loop sb-OUTER: sb0's denominator+recip complete mid-phase and
    overlap; the dn accumulator shrinks to ONE PSUM bank, giving the
    att psum pool back its 3rd buffer.
  * adj prefetch as full s-rows ([128, 2048B] DMA lines, was 1KB);
    y8/x8 as full 1KB rows (was 512B halves): fp8 halved line sizes
    and the projections started outrunning the input stream.
  * qkv PSUM pool 3 -> 4 bufs (K-phase evac backpressure).
  NOTE: the device power-throttles under sustained benchmarking
  (throttle_active 13us..212us run-to-run); back-to-back runs read
  ~5-20us slower than a cool run.

v10 = v9 (214us) + fp8 K/Q projections:
  * HW shows matmul issue cadence is 216ns/instruction in EVERY phase
    (fp8 DoubleRow streams the moving tensor at the same 1 col/cycle as
    bf16 -- the 2x is the doubled contraction depth per instruction, so
    the win is the halved instruction count). K and Q projections move
    to fp8 DoubleRow: x and a second copy of y are fp8 on host; Wq/Wk
    are fp8 at scale sqrt(1/32)*64 (all values normal in e4m3) and the
    evacuation ACT applies scale=1/64. V/O projections stay bf16 (fp8
    there pushes rel err past the 2e-2 gate; numpy sim: 1.774e-2 for
    this config vs 1.492e-2 for v9).
  * denominator matmuls defer by TWO exp-pairs (flush depth 2) so the
    PE never waits on the DVE->ACT exp chain.

v9 = v8 (236us) + denominator on the PE + DMA queue rebalance:
  * v8's 29us AV stall: the softmax denominator accumulated on GpSimd
    (~1.2us per [128,512] add x32 = 37us backlog in a 28us att phase);
    the reduce->recip->DVE chain released the first AV matmul at 176us.
    v9 computes the denominator with ones-vector DoubleRow matmuls
    interleaved into the att phase (16 x ~110ns on the PE), recip on
    DVE, partition-broadcast on the now-idle GpSimd.
  * kp slot-0's AllGather-gated load moves to the scalar DMA queue so
    it stops head-of-line-blocking the sync queue; A/O loads issue
    kp1 -> vp -> wo (each gated later than the previous resolves).

v8 = v7 (295us sched) + fp8 attention core (HW 277us -> 236us):
  * Q/K/V/exp tiles are TRN fp8_e4m3 (max +-240; our values <16).
    sqrt(1/32) of the softmax norm is folded into BOTH Wq and Wk host-
    side so Q/K entries sit at std ~0.10 (comfortably normal in e4m3).
  * att and AV matmuls run in MatmulPerfMode.DoubleRow: one instruction
    contracts a PAIR of 128-deep k-planes (lhsT [128,2,M], rhs [128,2,N])
    at 0.5 cycles/row -> 2x PE throughput. numpy-simulated rel_fro
    1.48e-2 vs the 2e-2 gate (bf16 everywhere: 1.9e-3).
  * K/V pair exchanges + kT_all/v_all now fp8: half the ring bytes.
  * AV accumulates BOTH r-slots in one PSUM group (r innermost): the
    fp32 num_sb resident tile and its ACT-copy/DVE-add evacuations are
    gone; evac is a single DVE mul by rb into bf16 on_sb.
  * exp ACTs process tt-PAIRS ([128,1024] per op) to halve ACT
    per-op overhead in the (now 2x faster) att phase.
  * denominator partials accumulate on GpSimd from the fp8 ex tiles;
    per-sb partition_all_reduce + reciprocal issue as soon as that sb's
    last ex lands (4 att groups before phase end) so rb[sb=0] is ready
    when AV starts.
  * outT stores bf16 (host casts back to f32): halves the final store
    tail; adds ~1e-3 rel err in quadrature (negligible vs 1.5e-2).
All attention math runs in "transposed" space:
    KT[d,t]   = matmul(lhsT=Wk*sqN, rhs=yT)              (+bk*sqN per-part)
    V [t,d]   = matmul(lhsT=yT, rhs=Wv)                  (+bv via gpsimd bcast)
    QT[d,s]   = matmul(lhsT=Wq*sqN, rhs=xT)              (+bq*sqN per-part)
    attT[t,s] = matmul_f8dr(lhsT=KT, rhs=QT)  (+adjT via DVE, exp via ACT)
    numT[d,s] = matmul_f8dr(lhsT=V,  rhs=exp)  (PSUM, both slots accum)
    denom[s]  = GpSimd-accumulated exp + partition_all_reduce
    outT[d2,s]= matmul(lhsT=Wo, rhs=numT*recip(denom))   (+bo per-partition)
  softmax max-subtraction is skipped: logits are O(1) by construction.
"""
import sys

if "/opt/trn_rl_repo" not in sys.path:
    sys.path.insert(0, "/opt/trn_rl_repo")

import numpy as np
import ml_dtypes

import concourse.bass as bass
import concourse.bass_isa as bass_isa
import concourse.tile as tile
from concourse import bacc, mybir
from concourse.bass_utils import run_bass_kernel_spmd

P = 128
D = 1024
S = 2048
SC = 1024            # per-core query rows
TH = 1024            # per-core own K/V t-half
DC = D // P          # 8 feature chunks
SB = 512             # matmul moving free dim
NSB = SC // SB       # 2 s blocks
TTP = 4              # t-tiles (128) per 512-panel
NORM = 1.0 / 32.0
SQN = float(1.0 / np.sqrt(32.0))   # folded into both Wq and Wk
WS = 64.0                          # fp8 weight pre-scale (ACT undoes it)
GROUPS = [[0, 1], [2, 3], [4, 5], [6, 7]]

F32 = mybir.dt.float32
BF16 = mybir.dt.bfloat16
F8 = mybir.dt.float8e4
ID = mybir.ActivationFunctionType.Identity
EXP = mybir.ActivationFunctionType.Exp
DR = mybir.MatmulPerfMode.DoubleRow
BF16NP = ml_dtypes.bfloat16
F8NP = ml_dtypes.float8_e4m3

_CACHE = {}


def _mm(nc, ps, lhsT, rhs, start, stop, perf_mode=None):
    nc.tensor.matmul(ps, lhsT=lhsT, rhs=rhs, start=start, stop=stop,
                     perf_mode=perf_mode)


def build_nc():
    nc = bacc.Bacc("TRN2", target_bir_lowering=False, debug=False, num_devices=8)

    xT = nc.dram_tensor("xT", [D, SC], F8, kind="ExternalInput")
    yT = nc.dram_tensor("yT", [D, TH], BF16, kind="ExternalInput")  # own t-half
    yT8 = nc.dram_tensor("yT8", [D, TH], F8, kind="ExternalInput")  # for K proj
    adjT = nc.dram_tensor("adjT", [S, SC], BF16, kind="ExternalInput")
    # Wq/Wk pre-tiled on host: Wx_t[dt][p][c][col] = Wx[c*P+p, dt*P+col]
    Wq = nc.dram_tensor("Wq", [DC, P, DC, P], F8, kind="ExternalInput")
    Wk = nc.dram_tensor("Wk", [DC, P, DC, P], F8, kind="ExternalInput")
    # Wo in natural [d_k, d2] layout
    Wo = nc.dram_tensor("Wo", [D, D], BF16, kind="ExternalInput")
    # Wv pre-tiled as rhs: Wv_t[db][p][c][col] = Wv[c*P+p, db*SB+col]
    Wv = nc.dram_tensor("Wv", [2, P, DC, SB], BF16, kind="ExternalInput")
    bq = nc.dram_tensor("bq", [P, DC], F32, kind="ExternalInput")
    bk = nc.dram_tensor("bk", [P, DC], F32, kind="ExternalInput")
    bv = nc.dram_tensor("bv", [1, D], F32, kind="ExternalInput")
    bo = nc.dram_tensor("bo", [P, DC], F32, kind="ExternalInput")
    outT = nc.dram_tensor("outT", [D, SC], BF16, kind="ExternalOutput")

    # pair exchange tensors (fp8)
    kT_loc = nc.dram_tensor("kT_loc", [D, TH], F8)
    v_loc = nc.dram_tensor("v_loc", [TH, D], F8)
    kT_all = nc.dram_tensor("kT_all", [2, D, TH], F8)
    v_all = nc.dram_tensor("v_all", [2, TH, D], F8)

    xT_r = xT.rearrange("(c p) s -> p c s", p=P)
    yT_r = yT.rearrange("(c p) t -> p c t", p=P)
    yT8_r = yT8.rearrange("(c p) t -> p c t", p=P)
    Wo_r = Wo.rearrange("(c p) o -> p c o", p=P)
    kT_all_r = kT_all.rearrange("r (c p) t -> r p c t", p=P)
    v_all_r = v_all.rearrange("r (j p) d -> r p j d", p=P)

    with tile.TileContext(nc) as tc:
        with (
            nc.allow_low_precision(reason="fp8 attention keeps rel err ~1.5e-2"),
            tc.tile_pool(name="res", bufs=1) as res,
        ):
            # ---- resident tiles --------------------------------------
            QT_sb = res.tile([P, DC, SC], F8, name="QT_sb")
            # ones as a [128, 2, 128] fp8 lhsT: the denominator matmul
            # then writes all 128 output partitions (same PE cost, the
            # cost scales with output columns), so the result is already
            # partition-broadcast and recip feeds rb directly
            ones8 = res.tile([P, 2, P], F8, name="ones8")
            nc.vector.memset(ones8[:], 1.0)
            rb = res.tile([P, NSB, SB], F32, name="rb")
            bv_bc = res.tile([P, D], F32, name="bv_bc")
            bq_sb = res.tile([P, DC], F32, name="bq_sb")
            bk_sb = res.tile([P, DC], F32, name="bk_sb")
            bo_sb = res.tile([P, DC], F32, name="bo_sb")
            bv_sb = res.tile([1, D], F32, name="bv_sb")
            nc.scalar.dma_start(out=bk_sb[:], in_=bk[:])
            nc.scalar.dma_start(out=bv_sb[:], in_=bv[:])
            nc.scalar.dma_start(out=bq_sb[:], in_=bq[:])
            nc.scalar.dma_start(out=bo_sb[:], in_=bo[:])
            # bv broadcast issues AFTER the wk loads (below): its bv_sb
            # gate must not head-of-line-block the gpsimd DMA queue

            # hoisted pools: adj fully prefetched early; kp slot-0 loads
            # during the projections (self-gated on the K AllGather)
            with (
                tc.tile_pool(name="kp_pool", bufs=1) as kpp,
                tc.tile_pool(name="adj_pool", bufs=16) as adjp,
            ):
              kps = {}
              ats = {}

              def load_adj(r):
                # full s-rows: 2KB DMA lines (adj is the largest input)
                for lb in range(2):
                    for tt in range(TTP):
                        tg = (r * 2 + lb) * TTP + tt
                        at = adjp.tile([P, SC], BF16, name="at")
                        nc.sync.dma_start(
                            out=at[:], in_=adjT[tg * P : (tg + 1) * P, :]
                        )
                        ats[(r, lb, tt)] = at

              def load_kp(r, pool):
                kp = pool.tile([P, DC, TH], F8, name="kp")
                kps[r] = kp
                for c in range(DC):
                    nc.sync.dma_start(out=kp[:, c, :], in_=kT_all_r[r, :, c, :])

              with (
                tc.tile_pool(name="qkv_in", bufs=1) as qkvp,
                tc.tile_pool(name="wk_pool", bufs=1) as wkp,
                tc.tile_pool(name="wq_pool", bufs=1) as wqp,
                tc.tile_pool(name="wv_pool", bufs=1) as wvp,
                tc.tile_pool(name="kt_out", bufs=4) as kto,
                tc.tile_pool(name="vt_out", bufs=7) as vto,
                tc.tile_pool(name="qkv_ps", bufs=4, space="PSUM") as qps,
              ):
                yT_sb = qkvp.tile([P, DC, TH], BF16, name="yT_sb")
                y8_sb = qkvp.tile([P, DC, TH], F8, name="y8_sb")
                xT_sb = qkvp.tile([P, DC, SC], F8, name="xT_sb")
                wv_t = [wvp.tile([P, DC, SB], BF16, name=f"wv{i}") for i in range(2)]
                wk_t = [wkp.tile([P, DC, P], F8, name=f"wk{i}") for i in range(DC)]
                wq_t = [wqp.tile([P, DC, P], F8, name=f"wq{i}") for i in range(DC)]

                # ---- phase K: KT(own half) = (1/64) Wk'^T y8^T + bk --
                # wk on the (idle) gpsimd queue, y8 on sync: the 2MB of
                # K-phase inputs arrive over two queues in parallel
                nc.gpsimd.dma_start(out=wk_t[0][:], in_=Wk[0])
                for c in range(DC):
                    nc.sync.dma_start(out=y8_sb[:, c, :], in_=yT8_r[:, c, :])
                for dt in range(1, DC):
                    nc.gpsimd.dma_start(out=wk_t[dt][:], in_=Wk[dt])
                for tb in range(NSB):
                    for dt in range(DC):
                        ps = qps.tile([P, SB], F32, name="k_ps", tag="qkvps")
                        for j in range(DC // 2):
                            _mm(
                                nc, ps[:],
                                wk_t[dt][:, 2 * j : 2 * j + 2, :],
                                y8_sb[:, 2 * j : 2 * j + 2,
                                      tb * SB : (tb + 1) * SB],
                                j == 0, j == DC // 2 - 1,
                                perf_mode=DR,
                            )
                        kt = kto.tile([P, SB], F8, name="kt")
                        nc.scalar.activation(
                            out=kt[:], in_=ps[:], func=ID, scale=1.0 / WS,
                            bias=bk_sb[:, dt : dt + 1],
                        )
                        # store via the (idle) GpSimd queue: ACT evac +
                        # store on one queue is 1278ns/group vs 864ns PE
                        nc.gpsimd.dma_start(
                            out=kT_loc[dt * P : (dt + 1) * P,
                                       tb * SB : (tb + 1) * SB],
                            in_=kt[:],
                        )
                nc.gpsimd.partition_broadcast(bv_bc[:], bv_sb[0:1, :], channels=P)
                nc.gpsimd.collective_compute(
                    "AllGather", mybir.AluOpType.bypass,
                    replica_groups=GROUPS,
                    ins=[kT_loc[:]], outs=[kT_all[:]],
                )
                # wq on the ACT queue: issues after the kt stores, so the
                # store descriptors win the HW rings during phase K
                # wq + x8 ride the scalar queue (idle during V): the
                # sync queue carries ~6MB ahead of them at ~150GB/s and
                # was starving the Q phase
                for dt in range(DC):
                    nc.scalar.dma_start(out=wq_t[dt][:], in_=Wq[dt])
                for c in range(DC):
                    nc.scalar.dma_start(out=xT_sb[:, c, :], in_=xT_r[:, c, :])

                # remaining input streams, in need order; adj rides the
                # quiet early window; kp slot 0 self-gates on AllGather K
                for db in range(2):
                    nc.sync.dma_start(out=wv_t[db][:], in_=Wv[db])
                for c in range(DC):
                    nc.sync.dma_start(out=yT_sb[:, c, :], in_=yT_r[:, c, :])
                load_adj(0)
                load_adj(1)
                # kp slot 0 on the SCALAR queue: it gates on the K
                # AllGather, and would head-of-line-block the sync
                # queue's A/O loads (kp1/vp/wo) if issued there
                kp = kpp.tile([P, DC, TH], F8, name="kp")
                kps[0] = kp
                for c in range(DC):
                    nc.scalar.dma_start(out=kp[:, c, :], in_=kT_all_r[0, :, c, :])

                # ---- phase V: V(own half) = y Wv + bv ----------------
                for tt in range(TH // P):
                    for db in range(2):
                        ps = qps.tile([P, SB], F32, name="v_ps", tag="qkvps")
                        for c in range(DC):
                            _mm(
                                nc, ps[:],
                                yT_sb[:, c, tt * P : (tt + 1) * P],
                                wv_t[db][:, c, :],
                                c == 0, c == DC - 1,
                            )
                        vt = vto.tile([P, SB], F8, name="vt")
                        nc.vector.tensor_add(
                            vt[:], ps[:], bv_bc[:, db * SB : (db + 1) * SB]
                        )
                        nc.gpsimd.dma_start(
                            out=v_loc[tt * P : (tt + 1) * P,
                                      db * SB : (db + 1) * SB],
                            in_=vt[:],
                        )
                nc.gpsimd.collective_compute(
                    "AllGather", mybir.AluOpType.bypass,
                    replica_groups=GROUPS,
                    ins=[v_loc[:]], outs=[v_all[:]],
                )

                # ---- phase Q: QT = (1/64) Wq'^T x8^T + bq ------------
                for dt in range(DC):
                    for sb in range(NSB):
                        ps = qps.tile([P, SB], F32, name="q_ps", tag="qkvps")
                        for j in range(DC // 2):
                            _mm(
                                nc, ps[:],
                                wq_t[dt][:, 2 * j : 2 * j + 2, :],
                                xT_sb[:, 2 * j : 2 * j + 2,
                                      sb * SB : (sb + 1) * SB],
                                j == 0, j == DC // 2 - 1,
                                perf_mode=DR,
                            )
                        nc.scalar.activation(
                            out=QT_sb[:, dt, sb * SB : (sb + 1) * SB],
                            in_=ps[:], func=ID, scale=1.0 / WS,
                            bias=bq_sb[:, dt : dt + 1],
                        )

              # ---- phase A + O share pools (no teardown barrier) -----
              with (
                tc.tile_pool(name="kp1_pool", bufs=1) as kpp1,
                tc.tile_pool(name="vp_pool", bufs=2) as vpp,
                tc.tile_pool(name="wo_pool", bufs=1) as wop,
                tc.tile_pool(name="on_pool", bufs=1) as onp,
                tc.tile_pool(name="exp_pool", bufs=8) as expp,
                tc.tile_pool(name="tmp_pool", bufs=3) as tmpp,
                tc.tile_pool(name="ot_pool", bufs=3) as otp,
                tc.tile_pool(name="aps", bufs=3, space="PSUM") as aps,
                tc.tile_pool(name="nps", bufs=4, space="PSUM") as npsp,
                tc.tile_pool(name="dnps", bufs=1, space="PSUM") as dnp,
              ):
                # sync-queue order: kp1 (K-AG gate, resolves first),
                # vp (V-AG gate), wo (ungated, rides behind)
                load_kp(1, kpp1)
                vps = {}
                for r in range(2):
                    vp = vpp.tile([P, TH // P, D], F8, name="vp")
                    vps[r] = vp
                    for j in range(TH // P):
                        nc.sync.dma_start(out=vp[:, j, :], in_=v_all_r[r, :, j, :])
                wo_t = wop.tile([P, DC, TH], BF16, name="wo")
                for c in range(DC):
                    nc.sync.dma_start(out=wo_t[:, c, :], in_=Wo_r[:, c, :])
                on_sb = onp.tile([P, DC, SC], BF16, name="on_sb")
                # denominator accumulator: ONE bank, reused per sb
                # (att loop is sb-outer so the sb groups are disjoint)
                dn = dnp.tile([P, SB], F32, name="dn")

                # att for BOTH slots (fp8 DoubleRow: c-pairs) before any
                # AV work; exp ACTs run on tt-PAIRS ([128,1024] per op);
                # denominator partials accumulate on GpSimd; per-sb
                # reduce+recip issues at that sb's last ex tile
                exs = {}
                pend_dn = []   # denominator matmuls, issued two tps late

                def flush_dn(keep=0):
                    while len(pend_dn) > keep:
                        pend_dn.pop(0)()

                def push_dn(r, lb, sb, tp, ex):
                    exsl = ex[:, 2 * tp : 2 * tp + 2, :]
                    st = r == 0 and lb == 0 and tp == 0
                    sp = r == 1 and lb == 1 and tp == TTP // 2 - 1
                    def go():
                        _mm(nc, dn[:], ones8[:], exsl, st, sp, perf_mode=DR)
                        if sp:
                            nc.vector.reciprocal_approx_fast(rb[:, sb, :], dn[:])
                    pend_dn.append(go)

                # sb-outer: sb0's denominator (and its fast recip)
                # complete halfway through the phase, fully overlapped
                for sb in range(NSB):
                    ssl = slice(sb * SB, (sb + 1) * SB)
                    for r in range(2):
                        kp = kps[r]
                        for lb in range(2):
                            ex = expp.tile([P, TTP, SB], F8, name="ex")
                            exs[(r, lb, sb)] = ex
                            for tp in range(TTP // 2):
                                tm = tmpp.tile([P, 2, SB], F32, name="tm")
                                for half in range(2):
                                    tt = 2 * tp + half
                                    att = aps.tile([P, SB], F32, name="att")
                                    for j in range(DC // 2):
                                        _mm(
                                            nc, att[:],
                                            kp[:, 2 * j : 2 * j + 2,
                                               lb * SB + tt * P
                                               : lb * SB + (tt + 1) * P],
                                            QT_sb[:, 2 * j : 2 * j + 2, ssl],
                                            j == 0, j == DC // 2 - 1,
                                            perf_mode=DR,
                                        )
                                    nc.vector.tensor_add(
                                        tm[:, half, :], att[:],
                                        ats[(r, lb, tt)][:, ssl],
                                    )
                                flush_dn(keep=2)   # 3-pair-old denom mm
                                nc.scalar.activation(
                                    out=ex[:, 2 * tp : 2 * tp + 2, :],
                                    in_=tm[:], func=EXP,
                                )
                                push_dn(r, lb, sb, tp, ex)
                flush_dn()

                # AV: both slots accumulate into ONE PSUM group (fp8
                # DoubleRow: tt-pairs); evac = single DVE mul by rb
                for sb in range(NSB):
                    ssl = slice(sb * SB, (sb + 1) * SB)
                    for dh in range(2):
                        nt = [
                            npsp.tile([P, SB], F32, name="np")
                            for _ in range(DC // 2)
                        ]
                        for r in range(2):
                            vp = vps[r]
                            for lb in range(2):
                                ex = exs[(r, lb, sb)]
                                for tp in range(TTP // 2):
                                    for d4 in range(DC // 2):
                                        _mm(
                                            nc, nt[d4][:],
                                            vp[:, lb * TTP + 2 * tp
                                               : lb * TTP + 2 * tp + 2,
                                               (dh * 4 + d4) * P
                                               : (dh * 4 + d4 + 1) * P],
                                            ex[:, 2 * tp : 2 * tp + 2, :],
                                            r == 0 and lb == 0 and tp == 0,
                                            r == 1 and lb == 1
                                            and tp == TTP // 2 - 1,
                                            perf_mode=DR,
                                        )
                        for d4 in range(DC // 2):
                            nc.vector.tensor_mul(
                                on_sb[:, dh * 4 + d4, ssl],
                                nt[d4][:],
                                rb[:, sb, :],
                            )

                # ---- phase O: out^T = Wo^T (numT*recip) + bo ---------
                # sb-outer: sb0 matmuls run while sb1's finalize completes
                for sb in range(NSB):
                    ssl = slice(sb * SB, (sb + 1) * SB)
                    for dt in range(DC):
                        po = aps.tile([P, SB], F32, name="att")
                        for c in range(DC):
                            _mm(
                                nc, po[:],
                                wo_t[:, c, dt * P : (dt + 1) * P],
                                on_sb[:, c, ssl],
                                c == 0, c == DC - 1,
                            )
                        ot = otp.tile([P, SB], BF16, name="ot")
                        nc.scalar.activation(
                            out=ot[:], in_=po[:], func=ID,
                            bias=bo_sb[:, dt : dt + 1],
                        )
                        nc.scalar.dma_start(
                            out=outT[dt * P : (dt + 1) * P,
                                     sb * SB : (sb + 1) * SB],
                            in_=ot[:],
                        )
    nc.compile()
    return nc


def _get_nc():
    if "nc" not in _CACHE:
        _CACHE["nc"] = build_nc()
    return _CACHE["nc"]


def _tile_lhs(W, dt=None):
    # [dt][p][c][col] = W[c*P+p, dt*P+col]
    return np.ascontiguousarray(
        W.reshape(DC, P, DC, P).transpose(2, 1, 0, 3).astype(dt or BF16NP)
    )


def kernel(x, y, adj, Wq, bq, Wk, bk, Wv, bv, Wo, bo, _trace=False):
    x = np.asarray(x, dtype=np.float32)
    y = np.asarray(y, dtype=np.float32)
    adj = np.asarray(adj, dtype=np.float32)
    Wq_h = _tile_lhs(np.asarray(Wq, np.float32) * (SQN * WS), F8NP)
    Wk_h = _tile_lhs(np.asarray(Wk, np.float32) * (SQN * WS), F8NP)
    Wo_h = np.ascontiguousarray(np.asarray(Wo, np.float32).astype(BF16NP))
    # Wv as rhs tiles: [db][p][c][col] = Wv[c*P+p, db*SB+col]
    Wv_h = np.ascontiguousarray(
        np.asarray(Wv, np.float32).reshape(DC, P, 2, SB)
        .transpose(2, 1, 0, 3).astype(BF16NP)
    )
    bq_h = np.ascontiguousarray((np.asarray(bq, np.float32) * SQN).reshape(DC, P).T)
    bk_h = np.ascontiguousarray((np.asarray(bk, np.float32) * SQN).reshape(DC, P).T)
    bo_h = np.ascontiguousarray(np.asarray(bo, np.float32).reshape(DC, P).T)
    bv_h = np.ascontiguousarray(np.asarray(bv, np.float32).reshape(1, D))

    in_maps = []
    for c in range(8):
        b, h = c // 2, c % 2
        ssl = slice(h * SC, (h + 1) * SC)
        in_maps.append(
            {
                "xT": np.ascontiguousarray(x[b, ssl, :].T.astype(F8NP)),
                "yT": np.ascontiguousarray(y[b, ssl, :].T.astype(BF16NP)),
                "yT8": np.ascontiguousarray(y[b, ssl, :].T.astype(F8NP)),
                "adjT": np.ascontiguousarray(adj[b, ssl, :].T.astype(BF16NP)),
                "Wq": Wq_h, "Wk": Wk_h, "Wv": Wv_h, "Wo": Wo_h,
                "bq": bq_h, "bk": bk_h, "bv": bv_h, "bo": bo_h,
            }
        )

    nc = _get_nc()
    res = run_bass_kernel_spmd(nc, in_maps, list(range(8)), trace=_trace)
    if _trace:
        _CACHE["last_exec_time_ns"] = res.exec_time_ns
        _CACHE["last_trace"] = (
            res.instructions_and_trace[1] if res.instructions_and_trace else None
        )

    out = np.empty((4, S, D), np.float32)
    for c in range(8):
        b, h = c // 2, c % 2
        out[b, h * SC : (h + 1) * SC, :] = res.results[c]["outT"].T.astype(np.float32)
    return out


# revision 57
# speedup vs baseline: 1.0184x; 1.0184x over previous
"""Fused single-head cross-attention on 8 TRN2 NeuronCores (Bass/Tile).

Problem: out = (softmax(norm * (xWq+bq)(yWk+bk)^T + adj) @ (yWv+bv)) Wo + bo
Shapes: x,y [4, 2048, 1024], adj [4, 2048, 2048], all weights [1024, 1024].

Sharding: data-parallel over (batch, seq-half) -> 8 shards. Core c handles
batch b=c//2, query rows h*1024..(h+1)*1024 (h=c%2). K/V projections are
split across the core pair (each computes its own t-half of K^T and V) and
exchanged with one pair-wise AllGather each.

v10.2 = v10 + pipeline/DMA polish (HW 210us at moderate throttle):
  * reciprocal -> reciprocal_approx_fast (0.7us vs 3.4us on DVE; 18
    correct bits vs denominators ~1e3 -- error contribution ~4e-6).
  * att loop sb-OUTER: sb0's denominator+recip complete mid-phase and
    overlap; the dn accumulator shrinks to ONE PSUM bank, giving the
    att psum pool back its 3rd buffer.
  * adj prefetch as full s-rows ([128, 2048B] DMA lines, was 1KB);
    y8/x8 as full 1KB rows (was 512B halves): fp8 halved line sizes
    and the projections started outrunning the input stream.
  * qkv PSUM pool 3 -> 4 bufs (K-phase evac backpressure).
  NOTE: the device power-throttles under sustained benchmarking
  (throttle_active 13us..212us run-to-run); back-to-back runs read
  ~5-20us slower than a cool run.

v10 = v9 (214us) + fp8 K/Q projections:
  * HW shows matmul issue cadence is 216ns/instruction in EVERY phase
    (fp8 DoubleRow streams the moving tensor at the same 1 col/cycle as
    bf16 -- the 2x is the doubled contraction depth per instruction, so
    the win is the halved instruction count). K and Q projections move
    to fp8 DoubleRow: x and a second copy of y are fp8 on host; Wq/Wk
    are fp8 at scale sqrt(1/32)*64 (all values normal in e4m3) and the
    evacuation ACT applies scale=1/64. V/O projections stay bf16 (fp8
    there pushes rel err past the 2e-2 gate; numpy sim: 1.774e-2 for
    this config vs 1.492e-2 for v9).
  * denominator matmuls defer by TWO exp-pairs (flush depth 2) so the
    PE never waits on the DVE->ACT exp chain.

v9 = v8 (236us) + denominator on the PE + DMA queue rebalance:
  * v8's 29us AV stall: the softmax denominator accumulated on GpSimd
    (~1.2us per [128,512] add x32 = 37us backlog in a 28us att phase);
    the reduce->recip->DVE chain released the first AV matmul at 176us.
    v9 computes the denominator with ones-vector DoubleRow matmuls
    interleaved into the att phase (16 x ~110ns on the PE), recip on
    DVE, partition-broadcast on the now-idle GpSimd.
  * kp slot-0's AllGather-gated load moves to the scalar DMA queue so
    it stops head-of-line-blocking the sync queue; A/O loads issue
    kp1 -> vp -> wo (each gated later than the previous resolves).

v8 = v7 (295us sched) + fp8 attention core (HW 277us -> 236us):
  * Q/K/V/exp tiles are TRN fp8_e4m3 (max +-240; our values <16).
    sqrt(1/32) of the softmax norm is folded into BOTH Wq and Wk host-
    side so Q/K entries sit at std ~0.10 (comfortably normal in e4m3).
  * att and AV matmuls run in MatmulPerfMode.DoubleRow: one instruction
    contracts a PAIR of 128-deep k-planes (lhsT [128,2,M], rhs [128,2,N])
    at 0.5 cycles/row -> 2x PE throughput. numpy-simulated rel_fro
    1.48e-2 vs the 2e-2 gate (bf16 everywhere: 1.9e-3).
  * K/V pair exchanges + kT_all/v_all now fp8: half the ring bytes.
  * AV accumulates BOTH r-slots in one PSUM group (r innermost): the
    fp32 num_sb resident tile and its ACT-copy/DVE-add evacuations are
    gone; evac is a single DVE mul by rb into bf16 on_sb.
  * exp ACTs process tt-PAIRS ([128,1024] per op) to halve ACT
    per-op overhead in the (now 2x faster) att phase.
  * denominator partials accumulate on GpSimd from the fp8 ex tiles;
    per-sb partition_all_reduce + reciprocal issue as soon as that sb's
    last ex lands (4 att groups before phase end) so rb[sb=0] is ready
    when AV starts.
  * outT stores bf16 (host casts back to f32): halves the final store
    tail; adds ~1e-3 rel err in quadrature (negligible vs 1.5e-2).
All attention math runs in "transposed" space:
    KT[d,t]   = matmul(lhsT=Wk*sqN, rhs=yT)              (+bk*sqN per-part)
    V [t,d]   = matmul(lhsT=yT, rhs=Wv)                  (+bv via gpsimd bcast)
    QT[d,s]   = matmul(lhsT=Wq*sqN, rhs=xT)              (+bq*sqN per-part)
    attT[t,s] = matmul_f8dr(lhsT=KT, rhs=QT)  (+adjT via DVE, exp via ACT)
    numT[d,s] = matmul_f8dr(lhsT=V,  rhs=exp)  (PSUM, both slots accum)
    denom[s]  = GpSimd-accumulated exp + partition_all_reduce
    outT[d2,s]= matmul(lhsT=Wo, rhs=numT*recip(denom))   (+bo per-partition)
  softmax max-subtraction is skipped: logits are O(1) by construction.
"""
import sys

if "/opt/trn_rl_repo" not in sys.path:
    sys.path.insert(0, "/opt/trn_rl_repo")

import numpy as np
import ml_dtypes

import concourse.bass as bass
import concourse.bass_isa as bass_isa
import concourse.tile as tile
from concourse import bacc, mybir
from concourse.bass_utils import run_bass_kernel_spmd

P = 128
D = 1024
S = 2048
SC = 1024            # per-core query rows
TH = 1024            # per-core own K/V t-half
DC = D // P          # 8 feature chunks
SB = 512             # matmul moving free dim
NSB = SC // SB       # 2 s blocks
TTP = 4              # t-tiles (128) per 512-panel
NORM = 1.0 / 32.0
SQN = float(1.0 / np.sqrt(32.0))   # folded into both Wq and Wk
WS = 64.0                          # fp8 weight pre-scale (ACT undoes it)
GROUPS = [[0, 1], [2, 3], [4, 5], [6, 7]]

F32 = mybir.dt.float32
BF16 = mybir.dt.bfloat16
F8 = mybir.dt.float8e4
ID = mybir.ActivationFunctionType.Identity
EXP = mybir.ActivationFunctionType.Exp
DR = mybir.MatmulPerfMode.DoubleRow
BF16NP = ml_dtypes.bfloat16
F8NP = ml_dtypes.float8_e4m3

_CACHE = {}


def _mm(nc, ps, lhsT, rhs, start, stop, perf_mode=None):
    nc.tensor.matmul(ps, lhsT=lhsT, rhs=rhs, start=start, stop=stop,
                     perf_mode=perf_mode)


def build_nc():
    nc = bacc.Bacc("TRN2", target_bir_lowering=False, debug=False, num_devices=8)

    xT = nc.dram_tensor("xT", [D, SC], F8, kind="ExternalInput")
    yT = nc.dram_tensor("yT", [D, TH], BF16, kind="ExternalInput")  # own t-half
    yT8 = nc.dram_tensor("yT8", [D, TH], F8, kind="ExternalInput")  # for K proj
    adjT = nc.dram_tensor("adjT", [S, SC], BF16, kind="ExternalInput")
    # Wq/Wk pre-tiled on host: Wx_t[dt][p][c][col] = Wx[c*P+p, dt*P+col]
    Wq = nc.dram_tensor("Wq", [DC, P, DC, P], F8, kind="ExternalInput")
    Wk = nc.dram_tensor("Wk", [DC, P, DC, P], F8, kind="ExternalInput")
    # Wo in natural [d_k, d2] layout
    Wo = nc.dram_tensor("Wo", [D, D], BF16, kind="ExternalInput")
    # Wv pre-tiled as rhs: Wv_t[db][p][c][col] = Wv[c*P+p, db*SB+col]
    Wv = nc.dram_tensor("Wv", [2, P, DC, SB], BF16, kind="ExternalInput")
    bq = nc.dram_tensor("bq", [P, DC], F32, kind="ExternalInput")
    bk = nc.dram_tensor("bk", [P, DC], F32, kind="ExternalInput")
    bv = nc.dram_tensor("bv", [1, D], F32, kind="ExternalInput")
    bo = nc.dram_tensor("bo", [P, DC], F32, kind="ExternalInput")
    outT = nc.dram_tensor("outT", [D, SC], BF16, kind="ExternalOutput")

    # pair exchange tensors (fp8)
    kT_loc = nc.dram_tensor("kT_loc", [D, TH], F8)
    v_loc = nc.dram_tensor("v_loc", [TH, D], F8)
    kT_all = nc.dram_tensor("kT_all", [2, D, TH], F8)
    v_all = nc.dram_tensor("v_all", [2, TH, D], F8)

    xT_r = xT.rearrange("(c p) s -> p c s", p=P)
    yT_r = yT.rearrange("(c p) t -> p c t", p=P)
    yT8_r = yT8.rearrange("(c p) t -> p c t", p=P)
    Wo_r = Wo.rearrange("(c p) o -> p c o", p=P)
    kT_all_r = kT_all.rearrange("r (c p) t -> r p c t", p=P)
    v_all_r = v_all.rearrange("r (j p) d -> r p j d", p=P)

    with tile.TileContext(nc) as tc:
        with (
            nc.allow_low_precision(reason="fp8 attention keeps rel err ~1.5e-2"),
            tc.tile_pool(name="res", bufs=1) as res,
        ):
            # ---- resident tiles --------------------------------------
            QT_sb = res.tile([P, DC, SC], F8, name="QT_sb")
            # ones as a [128, 2, 128] fp8 lhsT: the denominator matmul
            # then writes all 128 output partitions (same PE cost, the
            # cost scales with output columns), so the result is already
            # partition-broadcast and recip feeds rb directly
            ones8 = res.tile([P, 2, P], F8, name="ones8")
            nc.vector.memset(ones8[:], 1.0)
            rb = res.tile([P, NSB, SB], F32, name="rb")
            bv_bc = res.tile([P, D], F32, name="bv_bc")
            bq_sb = res.tile([P, DC], F32, name="bq_sb")
            bk_sb = res.tile([P, DC], F32, name="bk_sb")
            bo_sb = res.tile([P, DC], F32, name="bo_sb")
            bv_sb = res.tile([1, D], F32, name="bv_sb")
            nc.scalar.dma_start(out=bk_sb[:], in_=bk[:])
            nc.scalar.dma_start(out=bv_sb[:], in_=bv[:])
            nc.scalar.dma_start(out=bq_sb[:], in_=bq[:])
            nc.scalar.dma_start(out=bo_sb[:], in_=bo[:])
            nc.gpsimd.partition_broadcast(bv_bc[:], bv_sb[0:1, :], channels=P)

            # hoisted pools: adj fully prefetched early; kp slot-0 loads
            # during the projections (self-gated on the K AllGather)
            with (
                tc.tile_pool(name="kp_pool", bufs=1) as kpp,
                tc.tile_pool(name="adj_pool", bufs=16) as adjp,
            ):
              kps = {}
              ats = {}

              def load_adj(r):
                # full s-rows: 2KB DMA lines (adj is the largest input)
                for lb in range(2):
                    for tt in range(TTP):
                        tg = (r * 2 + lb) * TTP + tt
                        at = adjp.tile([P, SC], BF16, name="at")
                        nc.sync.dma_start(
                            out=at[:], in_=adjT[tg * P : (tg + 1) * P, :]
                        )
                        ats[(r, lb, tt)] = at

              def load_kp(r, pool):
                kp = pool.tile([P, DC, TH], F8, name="kp")
                kps[r] = kp
                for c in range(DC):
                    nc.sync.dma_start(out=kp[:, c, :], in_=kT_all_r[r, :, c, :])

              with (
                tc.tile_pool(name="qkv_in", bufs=1) as qkvp,
                tc.tile_pool(name="wk_pool", bufs=1) as wkp,
                tc.tile_pool(name="wq_pool", bufs=1) as wqp,
                tc.tile_pool(name="wv_pool", bufs=1) as wvp,
                tc.tile_pool(name="kt_out", bufs=4) as kto,
                tc.tile_pool(name="vt_out", bufs=7) as vto,
                tc.tile_pool(name="qkv_ps", bufs=5, space="PSUM") as qps,
              ):
                yT_sb = qkvp.tile([P, DC, TH], BF16, name="yT_sb")
                y8_sb = qkvp.tile([P, DC, TH], F8, name="y8_sb")
                xT_sb = qkvp.tile([P, DC, SC], F8, name="xT_sb")
                wv_t = [wvp.tile([P, DC, SB], BF16, name=f"wv{i}") for i in range(2)]
                wk_t = [wkp.tile([P, DC, P], F8, name=f"wk{i}") for i in range(DC)]
                wq_t = [wqp.tile([P, DC, P], F8, name=f"wq{i}") for i in range(DC)]

                # ---- phase K: KT(own half) = (1/64) Wk'^T y8^T + bk --
                nc.sync.dma_start(out=wk_t[0][:], in_=Wk[0])
                for c in range(DC):
                    nc.sync.dma_start(out=y8_sb[:, c, :], in_=yT8_r[:, c, :])
                for dt in range(1, DC):
                    nc.sync.dma_start(out=wk_t[dt][:], in_=Wk[dt])
                for tb in range(NSB):
                    for dt in range(DC):
                        ps = qps.tile([P, SB], F32, name="k_ps", tag="qkvps")
                        for j in range(DC // 2):
                            _mm(
                                nc, ps[:],
                                wk_t[dt][:, 2 * j : 2 * j + 2, :],
                                y8_sb[:, 2 * j : 2 * j + 2,
                                      tb * SB : (tb + 1) * SB],
                                j == 0, j == DC // 2 - 1,
                                perf_mode=DR,
                            )
                        kt = kto.tile([P, SB], F8, name="kt")
                        nc.scalar.activation(
                            out=kt[:], in_=ps[:], func=ID, scale=1.0 / WS,
                            bias=bk_sb[:, dt : dt + 1],
                        )
                        # store via the (idle) GpSimd queue: ACT evac +
                        # store on one queue is 1278ns/group vs 864ns PE
                        nc.gpsimd.dma_start(
                            out=kT_loc[dt * P : (dt + 1) * P,
                                       tb * SB : (tb + 1) * SB],
                            in_=kt[:],
                        )
                nc.gpsimd.collective_compute(
                    "AllGather", mybir.AluOpType.bypass,
                    replica_groups=GROUPS,
                    ins=[kT_loc[:]], outs=[kT_all[:]],
                )
                # wq on the ACT queue: issues after the kt stores, so the
                # store descriptors win the HW rings during phase K
                # wq + x8 ride the scalar queue (idle during V): the
                # sync queue carries ~6MB ahead of them at ~150GB/s and
                # was starving the Q phase
                for dt in range(DC):
                    nc.scalar.dma_start(out=wq_t[dt][:], in_=Wq[dt])
                for c in range(DC):
                    nc.scalar.dma_start(out=xT_sb[:, c, :], in_=xT_r[:, c, :])

                # remaining input streams, in need order; adj rides the
                # quiet early window; kp slot 0 self-gates on AllGather K
                for db in range(2):
                    nc.sync.dma_start(out=wv_t[db][:], in_=Wv[db])
                for c in range(DC):
                    nc.sync.dma_start(out=yT_sb[:, c, :], in_=yT_r[:, c, :])
                load_adj(0)
                load_adj(1)
                # kp slot 0 on the SCALAR queue: it gates on the K
                # AllGather, and would head-of-line-block the sync
                # queue's A/O loads (kp1/vp/wo) if issued there
                kp = kpp.tile([P, DC, TH], F8, name="kp")
                kps[0] = kp
                for c in range(DC):
                    nc.scalar.dma_start(out=kp[:, c, :], in_=kT_all_r[0, :, c, :])

                # ---- phase V: V(own half) = y Wv + bv ----------------
                for tt in range(TH // P):
                    for db in range(2):
                        ps = qps.tile([P, SB], F32, name="v_ps", tag="qkvps")
                        for c in range(DC):
                            _mm(
                                nc, ps[:],
                                yT_sb[:, c, tt * P : (tt + 1) * P],
                                wv_t[db][:, c, :],
                                c == 0, c == DC - 1,
                            )
                        vt = vto.tile([P, SB], F8, name="vt")
                        nc.vector.tensor_add(
                            vt[:], ps[:], bv_bc[:, db * SB : (db + 1) * SB]
                        )
                        nc.gpsimd.dma_start(
                            out=v_loc[tt * P : (tt + 1) * P,
                                      db * SB : (db + 1) * SB],
                            in_=vt[:],
                        )
                nc.gpsimd.collective_compute(
                    "AllGather", mybir.AluOpType.bypass,
                    replica_groups=GROUPS,
                    ins=[v_loc[:]], outs=[v_all[:]],
                )

                # ---- phase Q: QT = (1/64) Wq'^T x8^T + bq ------------
                for dt in range(DC):
                    for sb in range(NSB):
                        ps = qps.tile([P, SB], F32, name="q_ps", tag="qkvps")
                        for j in range(DC // 2):
                            _mm(
                                nc, ps[:],
                                wq_t[dt][:, 2 * j : 2 * j + 2, :],
                                xT_sb[:, 2 * j : 2 * j + 2,
                                      sb * SB : (sb + 1) * SB],
                                j == 0, j == DC // 2 - 1,
                                perf_mode=DR,
                            )
                        nc.scalar.activation(
                            out=QT_sb[:, dt, sb * SB : (sb + 1) * SB],
                            in_=ps[:], func=ID, scale=1.0 / WS,
                            bias=bq_sb[:, dt : dt + 1],
                        )

              # ---- phase A + O share pools (no teardown barrier) -----
              with (
                tc.tile_pool(name="kp1_pool", bufs=1) as kpp1,
                tc.tile_pool(name="vp_pool", bufs=2) as vpp,
                tc.tile_pool(name="wo_pool", bufs=1) as wop,
                tc.tile_pool(name="on_pool", bufs=1) as onp,
                tc.tile_pool(name="exp_pool", bufs=8) as expp,
                tc.tile_pool(name="tmp_pool", bufs=4) as tmpp,
                tc.tile_pool(name="ot_pool", bufs=3) as otp,
                tc.tile_pool(name="aps", bufs=3, space="PSUM") as aps,
                tc.tile_pool(name="nps", bufs=4, space="PSUM") as npsp,
                tc.tile_pool(name="dnps", bufs=1, space="PSUM") as dnp,
              ):
                # sync-queue order: kp1 (K-AG gate, resolves first),
                # vp (V-AG gate), wo (ungated, rides behind)
                load_kp(1, kpp1)
                vps = {}
                for r in range(2):
                    vp = vpp.tile([P, TH // P, D], F8, name="vp")
                    vps[r] = vp
                    for j in range(TH // P):
                        nc.sync.dma_start(out=vp[:, j, :], in_=v_all_r[r, :, j, :])
                wo_t = wop.tile([P, DC, TH], BF16, name="wo")
                for c in range(DC):
                    nc.sync.dma_start(out=wo_t[:, c, :], in_=Wo_r[:, c, :])
                on_sb = onp.tile([P, DC, SC], BF16, name="on_sb")
                # denominator accumulator: ONE bank, reused per sb
                # (att loop is sb-outer so the sb groups are disjoint)
                dn = dnp.tile([P, SB], F32, name="dn")

                # att for BOTH slots (fp8 DoubleRow: c-pairs) before any
                # AV work; exp ACTs run on tt-PAIRS ([128,1024] per op);
                # denominator partials accumulate on GpSimd; per-sb
                # reduce+recip issues at that sb's last ex tile
                exs = {}
                pend_dn = []   # denominator matmuls, issued two tps late

                def flush_dn(keep=0):
                    while len(pend_dn) > keep:
                        pend_dn.pop(0)()

                def push_dn(r, lb, sb, tp, ex):
                    exsl = ex[:, 2 * tp : 2 * tp + 2, :]
                    st = r == 0 and lb == 0 and tp == 0
                    sp = r == 1 and lb == 1 and tp == TTP // 2 - 1
                    def go():
                        _mm(nc, dn[:], ones8[:], exsl, st, sp, perf_mode=DR)
                        if sp:
                            nc.vector.reciprocal_approx_fast(rb[:, sb, :], dn[:])
                    pend_dn.append(go)

                # sb-outer: sb0's denominator (and its fast recip)
                # complete halfway through the phase, fully overlapped
                for sb in range(NSB):
                    ssl = slice(sb * SB, (sb + 1) * SB)
                    for r in range(2):
                        kp = kps[r]
                        for lb in range(2):
                            ex = expp.tile([P, TTP, SB], F8, name="ex")
                            exs[(r, lb, sb)] = ex
                            for tp in range(TTP // 2):
                                tm = tmpp.tile([P, 2, SB], F32, name="tm")
                                for half in range(2):
                                    tt = 2 * tp + half
                                    att = aps.tile([P, SB], F32, name="att")
                                    for j in range(DC // 2):
                                        _mm(
                                            nc, att[:],
                                            kp[:, 2 * j : 2 * j + 2,
                                               lb * SB + tt * P
                                               : lb * SB + (tt + 1) * P],
                                            QT_sb[:, 2 * j : 2 * j + 2, ssl],
                                            j == 0, j == DC // 2 - 1,
                                            perf_mode=DR,
                                        )
                                    nc.vector.tensor_add(
                                        tm[:, half, :], att[:],
                                        ats[(r, lb, tt)][:, ssl],
                                    )
                                flush_dn(keep=2)   # 3-pair-old denom mm
                                nc.scalar.activation(
                                    out=ex[:, 2 * tp : 2 * tp + 2, :],
                                    in_=tm[:], func=EXP,
                                )
                                push_dn(r, lb, sb, tp, ex)
                flush_dn()

                # AV: both slots accumulate into ONE PSUM group (fp8
                # DoubleRow: tt-pairs); evac = single DVE mul by rb
                for sb in range(NSB):
                    ssl = slice(sb * SB, (sb + 1) * SB)
                    for dh in range(2):
                        nt = [
                            npsp.tile([P, SB], F32, name="np")
                            for _ in range(DC // 2)
                        ]
                        for r in range(2):
                            vp = vps[r]
                            for lb in range(2):
                                ex = exs[(r, lb, sb)]
                                for tp in range(TTP // 2):
                                    for d4 in range(DC // 2):
                                        _mm(
                                            nc, nt[d4][:],
                                            vp[:, lb * TTP + 2 * tp
                                               : lb * TTP + 2 * tp + 2,
                                               (dh * 4 + d4) * P
                                               : (dh * 4 + d4 + 1) * P],
                                            ex[:, 2 * tp : 2 * tp + 2, :],
                                            r == 0 and lb == 0 and tp == 0,
                                            r == 1 and lb == 1
                                            and tp == TTP // 2 - 1,
                                            perf_mode=DR,
                                        )
                        for d4 in range(DC // 2):
                            nc.vector.tensor_mul(
                                on_sb[:, dh * 4 + d4, ssl],
                                nt[d4][:],
                                rb[:, sb, :],
                            )

                # ---- phase O: out^T = Wo^T (numT*recip) + bo ---------
                # sb-outer: sb0 matmuls run while sb1's finalize completes
                for sb in range(NSB):
                    ssl = slice(sb * SB, (sb + 1) * SB)
                    for dt in range(DC):
                        po = aps.tile([P, SB], F32, name="att")
                        for c in range(DC):
                            _mm(
                                nc, po[:],
                                wo_t[:, c, dt * P : (dt + 1) * P],
                                on_sb[:, c, ssl],
                                c == 0, c == DC - 1,
                            )
                        ot = otp.tile([P, SB], BF16, name="ot")
                        nc.scalar.activation(
                            out=ot[:], in_=po[:], func=ID,
                            bias=bo_sb[:, dt : dt + 1],
                        )
                        nc.scalar.dma_start(
                            out=outT[dt * P : (dt + 1) * P,
                                     sb * SB : (sb + 1) * SB],
                            in_=ot[:],
                        )
    nc.compile()
    return nc


def _get_nc():
    if "nc" not in _CACHE:
        _CACHE["nc"] = build_nc()
    return _CACHE["nc"]


def _tile_lhs(W, dt=None):
    # [dt][p][c][col] = W[c*P+p, dt*P+col]
    return np.ascontiguousarray(
        W.reshape(DC, P, DC, P).transpose(2, 1, 0, 3).astype(dt or BF16NP)
    )


def kernel(x, y, adj, Wq, bq, Wk, bk, Wv, bv, Wo, bo, _trace=False):
    x = np.asarray(x, dtype=np.float32)
    y = np.asarray(y, dtype=np.float32)
    adj = np.asarray(adj, dtype=np.float32)
    Wq_h = _tile_lhs(np.asarray(Wq, np.float32) * (SQN * WS), F8NP)
    Wk_h = _tile_lhs(np.asarray(Wk, np.float32) * (SQN * WS), F8NP)
    Wo_h = np.ascontiguousarray(np.asarray(Wo, np.float32).astype(BF16NP))
    # Wv as rhs tiles: [db][p][c][col] = Wv[c*P+p, db*SB+col]
    Wv_h = np.ascontiguousarray(
        np.asarray(Wv, np.float32).reshape(DC, P, 2, SB)
        .transpose(2, 1, 0, 3).astype(BF16NP)
    )
    bq_h = np.ascontiguousarray((np.asarray(bq, np.float32) * SQN).reshape(DC, P).T)
    bk_h = np.ascontiguousarray((np.asarray(bk, np.float32) * SQN).reshape(DC, P).T)
    bo_h = np.ascontiguousarray(np.asarray(bo, np.float32).reshape(DC, P).T)
    bv_h = np.ascontiguousarray(np.asarray(bv, np.float32).reshape(1, D))

    in_maps = []
    for c in range(8):
        b, h = c // 2, c % 2
        ssl = slice(h * SC, (h + 1) * SC)
        in_maps.append(
            {
                "xT": np.ascontiguousarray(x[b, ssl, :].T.astype(F8NP)),
                "yT": np.ascontiguousarray(y[b, ssl, :].T.astype(BF16NP)),
                "yT8": np.ascontiguousarray(y[b, ssl, :].T.astype(F8NP)),
                "adjT": np.ascontiguousarray(adj[b, ssl, :].T.astype(BF16NP)),
                "Wq": Wq_h, "Wk": Wk_h, "Wv": Wv_h, "Wo": Wo_h,
                "bq": bq_h, "bk": bk_h, "bv": bv_h, "bo": bo_h,
            }
        )

    nc = _get_nc()
    res = run_bass_kernel_spmd(nc, in_maps, list(range(8)), trace=_trace)
    if _trace:
        _CACHE["last_exec_time_ns"] = res.exec_time_ns
        _CACHE["last_trace"] = (
            res.instructions_and_trace[1] if res.instructions_and_trace else None
        )

    out = np.empty((4, S, D), np.float32)
    for c in range(8):
        b, h = c // 2, c % 2
        out[b, h * SC : (h + 1) * SC, :] = res.results[c]["outT"].T.astype(np.float32)
    return out


# revision 65
# speedup vs baseline: 1.0541x; 1.0350x over previous
"""Fused single-head cross-attention on 8 TRN2 NeuronCores (Bass/Tile).

Problem: out = (softmax(norm * (xWq+bq)(yWk+bk)^T + adj) @ (yWv+bv)) Wo + bo
Shapes: x,y [4, 2048, 1024], adj [4, 2048, 2048], all weights [1024, 1024].

Sharding: data-parallel over (batch, seq-half) -> 8 shards. Core c handles
batch b=c//2, query rows h*1024..(h+1)*1024 (h=c%2). K/V projections are
split across the core pair (each computes its own t-half of K^T and V) and
exchanged with one pair-wise AllGather each.

v10.2 = v10 + pipeline/DMA polish (HW 210us at moderate throttle):
  * reciprocal -> reciprocal_approx_fast (0.7us vs 3.4us on DVE; 18
    correct bits vs denominators ~1e3 -- error contribution ~4e-6).
  * att loop sb-OUTER: sb0's denominator+recip complete mid-phase and
    overlap; the dn accumulator shrinks to ONE PSUM bank, giving the
    att psum pool back its 3rd buffer.
  * adj prefetch as full s-rows ([128, 2048B] DMA lines, was 1KB);
    y8/x8 as full 1KB rows (was 512B halves): fp8 halved line sizes
    and the projections started outrunning the input stream.
  * qkv PSUM pool 3 -> 4 bufs (K-phase evac backpressure).
  NOTE: the device power-throttles under sustained benchmarking
  (throttle_active 13us..212us run-to-run); back-to-back runs read
  ~5-20us slower than a cool run.

v10 = v9 (214us) + fp8 K/Q projections:
  * HW shows matmul issue cadence is 216ns/instruction in EVERY phase
    (fp8 DoubleRow streams the moving tensor at the same 1 col/cycle as
    bf16 -- the 2x is the doubled contraction depth per instruction, so
    the win is the halved instruction count). K and Q projections move
    to fp8 DoubleRow: x and a second copy of y are fp8 on host; Wq/Wk
    are fp8 at scale sqrt(1/32)*64 (all values normal in e4m3) and the
    evacuation ACT applies scale=1/64. V/O projections stay bf16 (fp8
    there pushes rel err past the 2e-2 gate; numpy sim: 1.774e-2 for
    this config vs 1.492e-2 for v9).
  * denominator matmuls defer by TWO exp-pairs (flush depth 2) so the
    PE never waits on the DVE->ACT exp chain.

v9 = v8 (236us) + denominator on the PE + DMA queue rebalance:
  * v8's 29us AV stall: the softmax denominator accumulated on GpSimd
    (~1.2us per [128,512] add x32 = 37us backlog in a 28us att phase);
    the reduce->recip->DVE chain released the first AV matmul at 176us.
    v9 computes the denominator with ones-vector DoubleRow matmuls
    interleaved into the att phase (16 x ~110ns on the PE), recip on
    DVE, partition-broadcast on the now-idle GpSimd.
  * kp slot-0's AllGather-gated load moves to the scalar DMA queue so
    it stops head-of-line-blocking the sync queue; A/O loads issue
    kp1 -> vp -> wo (each gated later than the previous resolves).

v8 = v7 (295us sched) + fp8 attention core (HW 277us -> 236us):
  * Q/K/V/exp tiles are TRN fp8_e4m3 (max +-240; our values <16).
    sqrt(1/32) of the softmax norm is folded into BOTH Wq and Wk host-
    side so Q/K entries sit at std ~0.10 (comfortably normal in e4m3).
  * att and AV matmuls run in MatmulPerfMode.DoubleRow: one instruction
    contracts a PAIR of 128-deep k-planes (lhsT [128,2,M], rhs [128,2,N])
    at 0.5 cycles/row -> 2x PE throughput. numpy-simulated rel_fro
    1.48e-2 vs the 2e-2 gate (bf16 everywhere: 1.9e-3).
  * K/V pair exchanges + kT_all/v_all now fp8: half the ring bytes.
  * AV accumulates BOTH r-slots in one PSUM group (r innermost): the
    fp32 num_sb resident tile and its ACT-copy/DVE-add evacuations are
    gone; evac is a single DVE mul by rb into bf16 on_sb.
  * exp ACTs process tt-PAIRS ([128,1024] per op) to halve ACT
    per-op overhead in the (now 2x faster) att phase.
  * denominator partials accumulate on GpSimd from the fp8 ex tiles;
    per-sb partition_all_reduce + reciprocal issue as soon as that sb's
    last ex lands (4 att groups before phase end) so rb[sb=0] is ready
    when AV starts.
  * outT stores bf16 (host casts back to f32): halves the final store
    tail; adds ~1e-3 rel err in quadrature (negligible vs 1.5e-2).
All attention math runs in "transposed" space:
    KT[d,t]   = matmul(lhsT=Wk*sqN, rhs=yT)              (+bk*sqN per-part)
    V [t,d]   = matmul(lhsT=yT, rhs=Wv)                  (+bv via gpsimd bcast)
    QT[d,s]   = matmul(lhsT=Wq*sqN, rhs=xT)              (+bq*sqN per-part)
    attT[t,s] = matmul_f8dr(lhsT=KT, rhs=QT)  (+adjT via DVE, exp via ACT)
    numT[d,s] = matmul_f8dr(lhsT=V,  rhs=exp)  (PSUM, both slots accum)
    denom[s]  = GpSimd-accumulated exp + partition_all_reduce
    outT[d2,s]= matmul(lhsT=Wo, rhs=numT*recip(denom))   (+bo per-partition)
  softmax max-subtraction is skipped: logits are O(1) by construction.
"""
import sys

if "/opt/trn_rl_repo" not in sys.path:
    sys.path.insert(0, "/opt/trn_rl_repo")

import numpy as np
import ml_dtypes

import concourse.bass as bass
import concourse.bass_isa as bass_isa
import concourse.tile as tile
from concourse import bacc, mybir
from concourse.bass_utils import run_bass_kernel_spmd

P = 128
D = 1024
S = 2048
SC = 1024            # per-core query rows
TH = 1024            # per-core own K/V t-half
DC = D // P          # 8 feature chunks
SB = 512             # matmul moving free dim
NSB = SC // SB       # 2 s blocks
TTP = 4              # t-tiles (128) per 512-panel
NORM = 1.0 / 32.0
SQN = float(1.0 / np.sqrt(32.0))   # folded into both Wq and Wk
WS = 64.0                          # fp8 weight pre-scale (ACT undoes it)
GROUPS = [[0, 1], [2, 3], [4, 5], [6, 7]]

F32 = mybir.dt.float32
BF16 = mybir.dt.bfloat16
F8 = mybir.dt.float8e4
ID = mybir.ActivationFunctionType.Identity
EXP = mybir.ActivationFunctionType.Exp
DR = mybir.MatmulPerfMode.DoubleRow
BF16NP = ml_dtypes.bfloat16
F8NP = ml_dtypes.float8_e4m3

_CACHE = {}


def _mm(nc, ps, lhsT, rhs, start, stop, perf_mode=None):
    nc.tensor.matmul(ps, lhsT=lhsT, rhs=rhs, start=start, stop=stop,
                     perf_mode=perf_mode)


def build_nc():
    nc = bacc.Bacc("TRN2", target_bir_lowering=False, debug=False, num_devices=8)

    xT = nc.dram_tensor("xT", [D, SC], F8, kind="ExternalInput")
    yT = nc.dram_tensor("yT", [D, TH], BF16, kind="ExternalInput")  # own t-half
    # y for the K proj, host-packed in c-PAIRS so each DMA moves 2KB
    # lines (fp8 halved line sizes to 1KB and the sync queue dropped
    # to ~150GB/s): yT8[cp, p, i, t] = y.T[(2cp+i)*128+p, t]
    yT8 = nc.dram_tensor("yT8", [DC // 2, P, 2, TH], F8, kind="ExternalInput")
    adjT = nc.dram_tensor("adjT", [S, SC], BF16, kind="ExternalInput")
    # Wq/Wk pre-tiled on host: Wx_t[dt][p][c][col] = Wx[c*P+p, dt*P+col]
    Wq = nc.dram_tensor("Wq", [DC, P, DC, P], F8, kind="ExternalInput")
    # Wk host-packed in dt-PAIRS for 2KB DMA lines (see yT8)
    Wk = nc.dram_tensor("Wk", [DC // 2, P, 2, DC, P], F8, kind="ExternalInput")
    # Wo in natural [d_k, d2] layout
    Wo = nc.dram_tensor("Wo", [D, D], BF16, kind="ExternalInput")
    # Wv pre-tiled as rhs: Wv_t[db][p][c][col] = Wv[c*P+p, db*SB+col]
    Wv = nc.dram_tensor("Wv", [2, P, DC, SB], BF16, kind="ExternalInput")
    bq = nc.dram_tensor("bq", [P, DC], F32, kind="ExternalInput")
    bk = nc.dram_tensor("bk", [P, DC], F32, kind="ExternalInput")
    bv = nc.dram_tensor("bv", [1, D], F32, kind="ExternalInput")
    bo = nc.dram_tensor("bo", [P, DC], F32, kind="ExternalInput")
    outT = nc.dram_tensor("outT", [D, SC], BF16, kind="ExternalOutput")

    # pair exchange tensors (fp8)
    kT_loc = nc.dram_tensor("kT_loc", [D, TH], F8)
    v_loc = nc.dram_tensor("v_loc", [TH, D], F8)
    kT_all = nc.dram_tensor("kT_all", [2, D, TH], F8)
    v_all = nc.dram_tensor("v_all", [2, TH, D], F8)

    xT_r = xT.rearrange("(c p) s -> p c s", p=P)
    yT_r = yT.rearrange("(c p) t -> p c t", p=P)
    Wo_r = Wo.rearrange("(c p) o -> p c o", p=P)
    kT_all_r = kT_all.rearrange("r (c p) t -> r p c t", p=P)
    v_all_r = v_all.rearrange("r (j p) d -> r p j d", p=P)

    with tile.TileContext(nc) as tc:
        with (
            nc.allow_low_precision(reason="fp8 attention keeps rel err ~1.5e-2"),
            tc.tile_pool(name="res", bufs=1) as res,
        ):
            # ---- resident tiles --------------------------------------
            QT_sb = res.tile([P, DC, SC], F8, name="QT_sb")
            # ones as a [128, 2, 128] fp8 lhsT: the denominator matmul
            # then writes all 128 output partitions (same PE cost, the
            # cost scales with output columns), so the result is already
            # partition-broadcast and recip feeds rb directly
            ones8 = res.tile([P, 2, P], F8, name="ones8")
            nc.vector.memset(ones8[:], 1.0)
            rb = res.tile([P, NSB, SB], F32, name="rb")
            bv_bc = res.tile([P, D], F32, name="bv_bc")
            bq_sb = res.tile([P, DC], F32, name="bq_sb")
            bk_sb = res.tile([P, DC], F32, name="bk_sb")
            bo_sb = res.tile([P, DC], F32, name="bo_sb")
            bv_sb = res.tile([1, D], F32, name="bv_sb")
            nc.scalar.dma_start(out=bk_sb[:], in_=bk[:])
            nc.scalar.dma_start(out=bv_sb[:], in_=bv[:])
            nc.scalar.dma_start(out=bq_sb[:], in_=bq[:])
            nc.scalar.dma_start(out=bo_sb[:], in_=bo[:])
            nc.gpsimd.partition_broadcast(bv_bc[:], bv_sb[0:1, :], channels=P)

            # hoisted pools: adj fully prefetched early; kp slot-0 loads
            # during the projections (self-gated on the K AllGather)
            with (
                tc.tile_pool(name="kp_pool", bufs=1) as kpp,
                tc.tile_pool(name="adj_pool", bufs=16) as adjp,
            ):
              kps = {}
              ats = {}

              def load_adj(r):
                # full s-rows: 2KB DMA lines (adj is the largest input)
                for lb in range(2):
                    for tt in range(TTP):
                        tg = (r * 2 + lb) * TTP + tt
                        at = adjp.tile([P, SC], BF16, name="at")
                        nc.sync.dma_start(
                            out=at[:], in_=adjT[tg * P : (tg + 1) * P, :]
                        )
                        ats[(r, lb, tt)] = at

              def load_kp(r, pool):
                kp = pool.tile([P, DC, TH], F8, name="kp")
                kps[r] = kp
                for c in range(DC):
                    nc.sync.dma_start(out=kp[:, c, :], in_=kT_all_r[r, :, c, :])

              with (
                tc.tile_pool(name="qkv_in", bufs=1) as qkvp,
                tc.tile_pool(name="wk_pool", bufs=1) as wkp,
                tc.tile_pool(name="wq_pool", bufs=1) as wqp,
                tc.tile_pool(name="wv_pool", bufs=1) as wvp,
                tc.tile_pool(name="kt_out", bufs=4) as kto,
                tc.tile_pool(name="vt_out", bufs=7) as vto,
                tc.tile_pool(name="qkv_ps", bufs=4, space="PSUM") as qps,
              ):
                yT_sb = qkvp.tile([P, DC, TH], BF16, name="yT_sb")
                y8_sb = qkvp.tile([P, DC, TH], F8, name="y8_sb")
                xT_sb = qkvp.tile([P, DC, SC], F8, name="xT_sb")
                wv_t = [wvp.tile([P, DC, SB], BF16, name=f"wv{i}") for i in range(2)]
                wka = wkp.tile([P, DC, DC, P], F8, name="wka")
                wq_t = [wqp.tile([P, DC, P], F8, name=f"wq{i}") for i in range(DC)]

                # ---- phase K: KT(own half) = (1/64) Wk'^T y8^T + bk --
                # pair-packed 2KB-line DMAs for the phase-gating inputs
                nc.sync.dma_start(out=wka[:, 0:2, :, :], in_=Wk[0])
                for cp in range(DC // 2):
                    nc.sync.dma_start(
                        out=y8_sb[:, 2 * cp : 2 * cp + 2, :], in_=yT8[cp]
                    )
                for dp in range(1, DC // 2):
                    nc.sync.dma_start(
                        out=wka[:, 2 * dp : 2 * dp + 2, :, :], in_=Wk[dp]
                    )
                for tb in range(NSB):
                    for dt in range(DC):
                        ps = qps.tile([P, SB], F32, name="k_ps", tag="qkvps")
                        for j in range(DC // 2):
                            _mm(
                                nc, ps[:],
                                wka[:, dt, 2 * j : 2 * j + 2, :],
                                y8_sb[:, 2 * j : 2 * j + 2,
                                      tb * SB : (tb + 1) * SB],
                                j == 0, j == DC // 2 - 1,
                                perf_mode=DR,
                            )
                        kt = kto.tile([P, SB], F8, name="kt")
                        nc.scalar.activation(
                            out=kt[:], in_=ps[:], func=ID, scale=1.0 / WS,
                            bias=bk_sb[:, dt : dt + 1],
                        )
                        # store via the (idle) GpSimd queue: ACT evac +
                        # store on one queue is 1278ns/group vs 864ns PE
                        nc.gpsimd.dma_start(
                            out=kT_loc[dt * P : (dt + 1) * P,
                                       tb * SB : (tb + 1) * SB],
                            in_=kt[:],
                        )
                nc.gpsimd.collective_compute(
                    "AllGather", mybir.AluOpType.bypass,
                    replica_groups=GROUPS,
                    ins=[kT_loc[:]], outs=[kT_all[:]],
                )
                # wq on the ACT queue: issues after the kt stores, so the
                # store descriptors win the HW rings during phase K
                # wq + x8 ride the scalar queue (idle during V): the
                # sync queue carries ~6MB ahead of them at ~150GB/s and
                # was starving the Q phase
                for dt in range(DC):
                    nc.scalar.dma_start(out=wq_t[dt][:], in_=Wq[dt])
                for c in range(DC):
                    nc.scalar.dma_start(out=xT_sb[:, c, :], in_=xT_r[:, c, :])

                # remaining input streams, in need order; adj rides the
                # quiet early window; kp slot 0 self-gates on AllGather K
                for db in range(2):
                    nc.sync.dma_start(out=wv_t[db][:], in_=Wv[db])
                for c in range(DC):
                    nc.sync.dma_start(out=yT_sb[:, c, :], in_=yT_r[:, c, :])
                load_adj(0)
                load_adj(1)
                # kp slot 0 on the SCALAR queue: it gates on the K
                # AllGather, and would head-of-line-block the sync
                # queue's A/O loads (kp1/vp/wo) if issued there
                kp = kpp.tile([P, DC, TH], F8, name="kp")
                kps[0] = kp
                for c in range(DC):
                    nc.scalar.dma_start(out=kp[:, c, :], in_=kT_all_r[0, :, c, :])

                # ---- phase V: V(own half) = y Wv + bv ----------------
                for tt in range(TH // P):
                    for db in range(2):
                        ps = qps.tile([P, SB], F32, name="v_ps", tag="qkvps")
                        for c in range(DC):
                            _mm(
                                nc, ps[:],
                                yT_sb[:, c, tt * P : (tt + 1) * P],
                                wv_t[db][:, c, :],
                                c == 0, c == DC - 1,
                            )
                        vt = vto.tile([P, SB], F8, name="vt")
                        nc.vector.tensor_add(
                            vt[:], ps[:], bv_bc[:, db * SB : (db + 1) * SB]
                        )
                        nc.gpsimd.dma_start(
                            out=v_loc[tt * P : (tt + 1) * P,
                                      db * SB : (db + 1) * SB],
                            in_=vt[:],
                        )
                nc.gpsimd.collective_compute(
                    "AllGather", mybir.AluOpType.bypass,
                    replica_groups=GROUPS,
                    ins=[v_loc[:]], outs=[v_all[:]],
                )

                # ---- phase Q: QT = (1/64) Wq'^T x8^T + bq ------------
                for dt in range(DC):
                    for sb in range(NSB):
                        ps = qps.tile([P, SB], F32, name="q_ps", tag="qkvps")
                        for j in range(DC // 2):
                            _mm(
                                nc, ps[:],
                                wq_t[dt][:, 2 * j : 2 * j + 2, :],
                                xT_sb[:, 2 * j : 2 * j + 2,
                                      sb * SB : (sb + 1) * SB],
                                j == 0, j == DC // 2 - 1,
                                perf_mode=DR,
                            )
                        nc.scalar.activation(
                            out=QT_sb[:, dt, sb * SB : (sb + 1) * SB],
                            in_=ps[:], func=ID, scale=1.0 / WS,
                            bias=bq_sb[:, dt : dt + 1],
                        )

              # ---- phase A + O share pools (no teardown barrier) -----
              with (
                tc.tile_pool(name="kp1_pool", bufs=1) as kpp1,
                tc.tile_pool(name="vp_pool", bufs=2) as vpp,
                tc.tile_pool(name="wo_pool", bufs=1) as wop,
                tc.tile_pool(name="on_pool", bufs=1) as onp,
                tc.tile_pool(name="exp_pool", bufs=8) as expp,
                tc.tile_pool(name="tmp_pool", bufs=3) as tmpp,
                tc.tile_pool(name="ot_pool", bufs=3) as otp,
                tc.tile_pool(name="aps", bufs=3, space="PSUM") as aps,
                tc.tile_pool(name="nps", bufs=4, space="PSUM") as npsp,
                tc.tile_pool(name="dnps", bufs=1, space="PSUM") as dnp,
              ):
                # sync-queue order: kp1 (K-AG gate, resolves first),
                # vp (V-AG gate), wo (ungated, rides behind)
                load_kp(1, kpp1)
                vps = {}
                for r in range(2):
                    vp = vpp.tile([P, TH // P, D], F8, name="vp")
                    vps[r] = vp
                    for j in range(TH // P):
                        nc.sync.dma_start(out=vp[:, j, :], in_=v_all_r[r, :, j, :])
                wo_t = wop.tile([P, DC, TH], BF16, name="wo")
                for c in range(DC):
                    nc.sync.dma_start(out=wo_t[:, c, :], in_=Wo_r[:, c, :])
                on_sb = onp.tile([P, DC, SC], BF16, name="on_sb")
                # denominator accumulator: ONE bank, reused per sb
                # (att loop is sb-outer so the sb groups are disjoint)
                dn = dnp.tile([P, SB], F32, name="dn")

                # att for BOTH slots (fp8 DoubleRow: c-pairs) before any
                # AV work; exp ACTs run on tt-PAIRS ([128,1024] per op);
                # denominator partials accumulate on GpSimd; per-sb
                # reduce+recip issues at that sb's last ex tile
                exs = {}
                pend_dn = []   # denominator matmuls, issued two tps late

                def flush_dn(keep=0):
                    while len(pend_dn) > keep:
                        pend_dn.pop(0)()

                def push_dn(r, lb, sb, tp, ex):
                    exsl = ex[:, 2 * tp : 2 * tp + 2, :]
                    st = r == 0 and lb == 0 and tp == 0
                    sp = r == 1 and lb == 1 and tp == TTP // 2 - 1
                    def go():
                        _mm(nc, dn[:], ones8[:], exsl, st, sp, perf_mode=DR)
                        if sp:
                            nc.vector.reciprocal_approx_fast(rb[:, sb, :], dn[:])
                    pend_dn.append(go)

                # sb-outer: sb0's denominator (and its fast recip)
                # complete halfway through the phase, fully overlapped
                for sb in range(NSB):
                    ssl = slice(sb * SB, (sb + 1) * SB)
                    for r in range(2):
                        kp = kps[r]
                        for lb in range(2):
                            ex = expp.tile([P, TTP, SB], F8, name="ex")
                            exs[(r, lb, sb)] = ex
                            for tp in range(TTP // 2):
                                tm = tmpp.tile([P, 2, SB], F32, name="tm")
                                for half in range(2):
                                    tt = 2 * tp + half
                                    att = aps.tile([P, SB], F32, name="att")
                                    for j in range(DC // 2):
                                        _mm(
                                            nc, att[:],
                                            kp[:, 2 * j : 2 * j + 2,
                                               lb * SB + tt * P
                                               : lb * SB + (tt + 1) * P],
                                            QT_sb[:, 2 * j : 2 * j + 2, ssl],
                                            j == 0, j == DC // 2 - 1,
                                            perf_mode=DR,
                                        )
                                    nc.vector.tensor_add(
                                        tm[:, half, :], att[:],
                                        ats[(r, lb, tt)][:, ssl],
                                    )
                                flush_dn(keep=2)   # 3-pair-old denom mm
                                nc.scalar.activation(
                                    out=ex[:, 2 * tp : 2 * tp + 2, :],
                                    in_=tm[:], func=EXP,
                                )
                                push_dn(r, lb, sb, tp, ex)
                flush_dn()

                # AV: both slots accumulate into ONE PSUM group (fp8
                # DoubleRow: tt-pairs); evac = single DVE mul by rb
                for sb in range(NSB):
                    ssl = slice(sb * SB, (sb + 1) * SB)
                    for dh in range(2):
                        nt = [
                            npsp.tile([P, SB], F32, name="np")
                            for _ in range(DC // 2)
                        ]
                        for r in range(2):
                            vp = vps[r]
                            for lb in range(2):
                                ex = exs[(r, lb, sb)]
                                for tp in range(TTP // 2):
                                    for d4 in range(DC // 2):
                                        _mm(
                                            nc, nt[d4][:],
                                            vp[:, lb * TTP + 2 * tp
                                               : lb * TTP + 2 * tp + 2,
                                               (dh * 4 + d4) * P
                                               : (dh * 4 + d4 + 1) * P],
                                            ex[:, 2 * tp : 2 * tp + 2, :],
                                            r == 0 and lb == 0 and tp == 0,
                                            r == 1 and lb == 1
                                            and tp == TTP // 2 - 1,
                                            perf_mode=DR,
                                        )
                        for d4 in range(DC // 2):
                            nc.vector.tensor_mul(
                                on_sb[:, dh * 4 + d4, ssl],
                                nt[d4][:],
                                rb[:, sb, :],
                            )

                # ---- phase O: out^T = Wo^T (numT*recip) + bo ---------
                # sb-outer: sb0 matmuls run while sb1's finalize completes
                for sb in range(NSB):
                    ssl = slice(sb * SB, (sb + 1) * SB)
                    for dt in range(DC):
                        po = aps.tile([P, SB], F32, name="att")
                        for c in range(DC):
                            _mm(
                                nc, po[:],
                                wo_t[:, c, dt * P : (dt + 1) * P],
                                on_sb[:, c, ssl],
                                c == 0, c == DC - 1,
                            )
                        ot = otp.tile([P, SB], BF16, name="ot")
                        nc.scalar.activation(
                            out=ot[:], in_=po[:], func=ID,
                            bias=bo_sb[:, dt : dt + 1],
                        )
                        nc.scalar.dma_start(
                            out=outT[dt * P : (dt + 1) * P,
                                     sb * SB : (sb + 1) * SB],
                            in_=ot[:],
                        )
    nc.compile()
    return nc


def _get_nc():
    if "nc" not in _CACHE:
        _CACHE["nc"] = build_nc()
    return _CACHE["nc"]


def _tile_lhs(W, dt=None):
    # [dt][p][c][col] = W[c*P+p, dt*P+col]
    return np.ascontiguousarray(
        W.reshape(DC, P, DC, P).transpose(2, 1, 0, 3).astype(dt or BF16NP)
    )


def kernel(x, y, adj, Wq, bq, Wk, bk, Wv, bv, Wo, bo, _trace=False):
    x = np.asarray(x, dtype=np.float32)
    y = np.asarray(y, dtype=np.float32)
    adj = np.asarray(adj, dtype=np.float32)
    Wq_h = _tile_lhs(np.asarray(Wq, np.float32) * (SQN * WS), F8NP)
    # Wk pair-packed: [dp, p, i, c, col] = tile_lhs[2*dp+i, p, c, col]
    Wk_h = np.ascontiguousarray(
        _tile_lhs(np.asarray(Wk, np.float32) * (SQN * WS), F8NP)
        .reshape(DC // 2, 2, P, DC, P).transpose(0, 2, 1, 3, 4)
    )
    Wo_h = np.ascontiguousarray(np.asarray(Wo, np.float32).astype(BF16NP))
    # Wv as rhs tiles: [db][p][c][col] = Wv[c*P+p, db*SB+col]
    Wv_h = np.ascontiguousarray(
        np.asarray(Wv, np.float32).reshape(DC, P, 2, SB)
        .transpose(2, 1, 0, 3).astype(BF16NP)
    )
    bq_h = np.ascontiguousarray((np.asarray(bq, np.float32) * SQN).reshape(DC, P).T)
    bk_h = np.ascontiguousarray((np.asarray(bk, np.float32) * SQN).reshape(DC, P).T)
    bo_h = np.ascontiguousarray(np.asarray(bo, np.float32).reshape(DC, P).T)
    bv_h = np.ascontiguousarray(np.asarray(bv, np.float32).reshape(1, D))

    in_maps = []
    for c in range(8):
        b, h = c // 2, c % 2
        ssl = slice(h * SC, (h + 1) * SC)
        in_maps.append(
            {
                "xT": np.ascontiguousarray(x[b, ssl, :].T.astype(F8NP)),
                "yT": np.ascontiguousarray(y[b, ssl, :].T.astype(BF16NP)),
                "yT8": np.ascontiguousarray(
                    y[b, ssl, :].T.astype(F8NP)
                    .reshape(DC // 2, 2, P, TH).transpose(0, 2, 1, 3)
                ),
                "adjT": np.ascontiguousarray(adj[b, ssl, :].T.astype(BF16NP)),
                "Wq": Wq_h, "Wk": Wk_h, "Wv": Wv_h, "Wo": Wo_h,
                "bq": bq_h, "bk": bk_h, "bv": bv_h, "bo": bo_h,
            }
        )

    nc = _get_nc()
    res = run_bass_kernel_spmd(nc, in_maps, list(range(8)), trace=_trace)
    if _trace:
        _CACHE["last_exec_time_ns"] = res.exec_time_ns
        _CACHE["last_trace"] = (
            res.instructions_and_trace[1] if res.instructions_and_trace else None
        )

    out = np.empty((4, S, D), np.float32)
    for c in range(8):
        b, h = c // 2, c % 2
        out[b, h * SC : (h + 1) * SC, :] = res.results[c]["outT"].T.astype(np.float32)
    return out
